# revision 49
# baseline (speedup 1.0000x reference)
"""Trainium2 Bass kernel for nn_AttnBlock (B=1, C=128, H=32, W=128, 8 heads).

Sharding: one attention head per NeuronCore (8 heads / 8 cores). Each core
computes its head's q/k/v projections, the full 4096x4096 attention for that
head, and the final (buggy-but-faithful) W-axis projection for its 16-channel
output slab. Host gathers the 8 slabs into the (1, 128, 32, 128) output.

Math per core (head i):
  q/k in (d, L) bf16 layout via one PE matmul per 512-chunk (wqk packed lhsT)
  v in (L, 32) fp8e4 tiles: [1 | v(16) | 0(15)] -> the ones col makes the EV
  matmul accumulate the softmax denominator for free; 15 zero pad cols give
  the DoubleRow pair a 32B stride.
  S^T tile = k_j^T q (bf16 PE matmul, l_k on partitions, l_q free)
  exp: tiles alternate between ACT (func=Exp, scale=4, bias=-2 -> e4m3) and
  DVE (Schraudolph uint8 bit trick straight to e4m3 bits); both emit
  exp(4S-2) in fp8e4 -- the e^-2 cancels in the softmax normalization.
  The exp stage is the wall: ACT+DVE each spend ~1.1us per [128,1024] tile.
  EV: ONE fp8 DoubleRow matmul per (l_k tile pair, chunk): lhsT = v pair
  [128,2,32], rhs = et pair [128,2,512], acc[32,512] += both tiles' worth.
  DR runs at 0.5 cycles/col -- 4x fewer PE column-cycles than bf16 two-MM.
  Main loop is software-pipelined at l_k-tile granularity: EV(p) is emitted
  after S(2p+3) so the in-order PE queue never head-of-line blocks on the
  slower exp stage.
  Striped epilogue (emitted across the NEXT cp's j-loop): all 8 transposes
  of a cp go into ONE borrowed psum bank, one strided reciprocal covers the
  8 sums columns, one broadcast tensor_tensor normalizes all 8 o-blocks,
  then two [D,4W] projection groups. NOTE: only the FIRST bias matmul in a
  shared psum bank may carry start=True -- a second start re-marks the whole
  2KB zero region and the earlier writes get dropped on the next accumulate.
"""

import math as _math

import numpy as np

N_CORES = 8
C = 128
H = 32
W = 128
L = H * W  # 4096
F = 8  # heads
D = 16  # head dim
SCALE = 4.0  # sqrt(D); reference MULTIPLIES by it
SHIFT = 2.0  # exp(4S - SHIFT): keeps e4m3 in range; cancels in softmax norm
D1 = 18  # epilogue acc rows used: 1 sums row + 16 o rows (+1 id pad)
VW = 32  # v tile width: [1 | v(16) | zeros(15)]; 32B stride for DoubleRow
CHUNK = 512  # l_q chunk width
NCHUNK = L // CHUNK  # 8
NKT = L // 128  # 32 l_k tiles of 128
# Of every 32 l_k tiles, route this many through the DVE fast-exp
# (Schraudolph uint8->e4m3 bit trick); the rest go through ACT Exp.
DVE_N = 16
EXTRA_ACT = ()  # j values (mod 32) forced onto ACT despite the alternation
EPI_J = (6, 10, 16, 24)  # j-points for the striped epilogue stages
INTERLEAVE = False  # stagger kq/v prologue into cp0's loop
WARM_N = 0  # PE pstate warmup dummy matmuls
EPI_J_DEFAULT = None
OSB1_ACT = False  # second in-loop osb copy on ACT instead of DVE
ACC0_ACT = False  # acc chunk-0 evacuation on ACT instead of DVE
SCH_A = float(SCALE * (1 << 3) / _math.log(2))
SCH_B = float(7 * 8 - 0.5 - SHIFT * 8 / _math.log(2))
CBLOB_W = 744  # packed: wq|wk|bq|bk|wpT_f32r|bp|id18|ones|mshift|bp2

_CACHE = {}


def _build():
    import concourse.tile as tile
    from concourse import bacc, mybir

    f32 = mybir.dt.float32
    f32r = mybir.dt.float32r
    bf16 = mybir.dt.bfloat16
    f8 = mybir.dt.float8e4
    u8 = mybir.dt.uint8
    Exp = mybir.ActivationFunctionType.Exp
    DR = mybir.MatmulPerfMode.DoubleRow

    nc = bacc.Bacc("TRN2", target_bir_lowering=False, debug=False)

    x_d = nc.dram_tensor("x_cl", [C, L], bf16, kind="ExternalInput").ap()
    cb_d = nc.dram_tensor("cblob", [C, CBLOB_W], f32r, kind="ExternalInput").ap()
    wpb_d = nc.dram_tensor("wpbf", [W, W + 2 * D + VW + 204], bf16, kind="ExternalInput").ap()
    out_d = nc.dram_tensor("out", [D, L], f32, kind="ExternalOutput").ap()

    with tile.TileContext(nc) as tc:
        with (
            tc.tile_pool(name="consts", bufs=1) as consts,
            tc.tile_pool(name="qk", bufs=1) as qkp,
            tc.tile_pool(name="vp", bufs=1) as vp,
            tc.tile_pool(name="epool", bufs=16) as epool,
            tc.tile_pool(name="e0pool", bufs=2) as e0pool,
            tc.tile_pool(name="episb", bufs=6) as episb,
        ):
            # ---- all small constants arrive in ONE DMA ----
            wkvb = consts.tile([W, W + 2 * D + VW + 204], bf16)
            nc.sync.dma_start(out=wkvb, in_=wpb_d)

            cb = consts.tile([C, CBLOB_W], f32r)
            id_sb = cb[0:D1, 326:344]
            ones128 = cb[0:1, 344:472]
            ones16 = cb[0:1, 472:488]
            bp2_sb = cb[0:1, 488:744]  # [b_proj, b_proj] for the shared bias matmul
            x_sb = consts.tile([C, L], bf16)
            for cch in range(NCHUNK):
                eng = nc.sync if cch % 2 == 0 else nc.gpsimd
                eng.dma_start(
                    out=x_sb[:, cch * CHUNK : (cch + 1) * CHUNK],
                    in_=x_d[:, cch * CHUNK : (cch + 1) * CHUNK],
                )
                if cch == 1:
                    nc.sync.dma_start(out=cb, in_=cb_d)
            wpbf_sb = wkvb[:, 0:W]
            bv_sb = wkvb[0:1, W : W + VW]  # [1 | bv | 0] row (bf16)
            wvb_sb = wkvb[:, W + VW : W + VW + VW]  # [0 | wv | 0] (bf16)
            wqk_sb = wkvb[:, W + 2 * VW : W + 2 * VW + 64]
            ones128b = wkvb[0:1, W + 2 * VW + 64 : W + 2 * VW + 192]
            bq_sb = wkvb[0:D, 384:386].bitcast(f32)
            bk_sb = wkvb[0:D, 386:388].bitcast(f32)
            mshift = wkvb[0:C, 388:390].bitcast(f32)

            q_sb = qkp.tile([D, L], bf16)
            k_sb = qkp.tile([D, L], bf16)
            v_sb = vp.tile([C, VW * NKT], f8)  # [1 | v | 0] tiles, VW cols each

            # ---- psum pools: 6 (squads, 3-deep pipeline) + 2 (acc) = 8 banks;
            # prologue/epilogue psum tiles borrow squad-pool slots ----
            with (
                tc.tile_pool(name="ps_s", bufs=3, space="PSUM") as ps_s,
                tc.tile_pool(name="ps_acc", bufs=2, space="PSUM") as ps_acc,
            ):
                ps_epi = ps_s
                # ---- warm the ACT exp table while DMAs run ----
                dummy = episb.tile([1, 2], f32, tag="dummy")
                nc.scalar.activation(out=dummy[:], in_=ones128[:, 0:2], func=Exp)
                # ---- warm the PE pstate ramp during the DMA wait: ~3us of
                # back-to-back dummy matmuls on (uninitialized) sbuf scratch;
                # results go to a psum tile nobody reads ----
                if WARM_N:
                    wsrc = episb.tile([1, 192], bf16, tag="wsrc")
                    nc.gpsimd.memset(wsrc[:], 0.0)
                    wps = ps_acc.tile([64, 128], f32, tag="acc")
                    for _w in range(WARM_N):
                        nc.tensor.matmul(
                            wps[:], wsrc[:, 0:64], wsrc[:, 64:192],
                            start=True, stop=True, skip_group_check=True,
                        )

                Ident = mybir.ActivationFunctionType.Identity

                def emit_k(cch, on_act):
                    # k chunks are needed up front (every cp iterates over all
                    # l_k tiles): alternate the evacuation across ACT/DVE so
                    # neither engine serializes the prologue
                    sl = slice(cch * CHUNK, (cch + 1) * CHUNK)
                    kps = ps_epi.tile([D, CHUNK], f32, tag="squad")
                    nc.tensor.matmul(
                        kps[:], wqk_sb[:, 32:48], x_sb[:, sl], start=True, stop=True
                    )
                    if on_act:
                        nc.scalar.activation(
                            out=k_sb[:, sl], in_=kps[:], func=Ident, bias=bk_sb[:]
                        )
                    else:
                        nc.vector.tensor_scalar_add(k_sb[:, sl], kps[:], bk_sb[:])

                def emit_q(cch, pair=False):
                    # q chunk c feeds only cp c//2 -> computed lazily in the
                    # previous cp's loop on ACT (which has spare slots there);
                    # pair=True does two chunks with a single wide evacuation
                    n = 2 if pair else 1
                    sl = slice(cch * CHUNK, (cch + n) * CHUNK)
                    qps = ps_epi.tile([D, n * CHUNK], f32, tag="squad")
                    for u in range(n):
                        nc.tensor.matmul(
                            qps[:, u * CHUNK : (u + 1) * CHUNK], wqk_sb[:, 0:16],
                            x_sb[:, (cch + u) * CHUNK : (cch + u + 1) * CHUNK],
                            start=True, stop=True, skip_group_check=True,
                        )
                    nc.scalar.activation(
                        out=q_sb[:, sl], in_=qps[:], func=Ident, bias=bq_sb[:]
                    )

                def emit_v_group_mms(g):
                    # v tiles 8g..8g+7 (uses x chunks 2g, 2g+1)
                    vps = ps_epi.tile([C, 8 * VW], f32, tag="squad")
                    for u in range(8):
                        t = 8 * g + u
                        vsl = slice(u * VW, (u + 1) * VW)
                        nc.tensor.matmul(
                            vps[:, vsl], ones128b[:], bv_sb[:],
                            start=True, stop=False, skip_group_check=True,
                        )
                        nc.tensor.matmul(
                            vps[:, vsl], x_sb[:, t * 128 : (t + 1) * 128], wvb_sb[:],
                            start=False, stop=True, skip_group_check=True,
                        )
                    return vps

                def emit_v_group_copy(g, vps):
                    nc.vector.tensor_copy(
                        v_sb[:, g * 8 * VW : (g + 1) * 8 * VW], vps[:]
                    )

                emit_k(0, on_act=False)
                emit_q(0, pair=True)
                for _cch in range(1, NCHUNK):
                    emit_k(_cch, on_act=(_cch % 2 == 1))
                    if _cch % 2 == 1:
                        emit_v_group_copy(_cch // 2, emit_v_group_mms(_cch // 2))

                # Striped epilogue: stages are emitted at different j's of
                # the NEXT cp's loop so engines never head-of-line block on
                # cross-engine inputs; borrowed psum slots are held briefly.
                def epi_transposes(acc_sb):
                    # all 8 128-blocks transpose into ONE borrowed psum bank
                    tbank = ps_epi.tile([128, 8 * D1], f32, tag="squad")
                    for s in range(8):
                        nc.tensor.matmul(
                            tbank[:, s * D1 : (s + 1) * D1],
                            acc_sb[:, s * 128 : (s + 1) * 128], id_sb[:],
                            start=True, stop=True, skip_group_check=True,
                        )
                    return tbank

                def epi_norm(tbank):
                    # one strided reciprocal over the 8 sums cols, then one
                    # broadcast-multiply producing all 8 normalized o blocks
                    recip8 = episb.tile([128, 8, 1], f32, tag="recip")
                    sums8 = tbank[:].rearrange("p (s w) -> p s w", w=D1)[:, :, 0:1]
                    nc.vector.reciprocal(recip8[:], sums8)
                    onorm = episb.tile([128, 8, D], bf16, tag="onorm")
                    o8 = tbank[:].rearrange("p (s w) -> p s w", w=D1)[:, :, 1 : D + 1]
                    nc.vector.tensor_tensor(
                        out=onorm[:], in0=o8,
                        in1=recip8[:].broadcast_to([128, 8, D]),
                        op=mybir.AluOpType.mult,
                    )
                    return onorm

                def epi_proj(cp, half, onorm, act_assist=False):
                    # half = 0/1: blocks 4*half..4*half+3 -> one [D, 4W] psum.
                    # ONLY the first bias matmul carries start=True: a second
                    # start in the same bank re-marks the whole zero region
                    # and the earlier bias is dropped on the next write.
                    pps = ps_epi.tile([D, 4 * W], f32, tag="squad")
                    for i in range(2):
                        nc.tensor.matmul(
                            pps[:, 2 * i * W : (2 * i + 2) * W], ones16[:],
                            bp2_sb[:], start=(i == 0), stop=False,
                            skip_group_check=True,
                        )
                    for i in range(4):
                        s = 4 * half + i
                        nc.tensor.matmul(
                            pps[:, i * W : (i + 1) * W],
                            onorm[:, s, :], wpbf_sb[:],
                            start=False, stop=(i == 3), skip_group_check=True,
                        )
                    osb = episb.tile([D, 4 * W], f32, tag="osb")
                    if act_assist:
                        nc.scalar.copy(osb[:], pps[:])
                    else:
                        nc.vector.tensor_copy(osb[:], pps[:])
                    h0 = 8 * cp + 4 * half
                    nc.sync.dma_start(
                        out=out_d[:, h0 * W : (h0 + 4) * W], in_=osb[:]
                    )

                # ---- main attention loop: chunk pairs, epilogues deferred ----
                pending = None  # (cp, acc_sb) awaiting epilogue emission
                for cp in range(NCHUNK // 2):
                    c0 = 2 * cp
                    sl0 = slice(c0 * CHUNK, (c0 + 1) * CHUNK)
                    sl1 = slice((c0 + 1) * CHUNK, (c0 + 2) * CHUNK)
                    acc0 = ps_acc.tile([VW, CHUNK], f32, tag="acc")
                    acc1 = ps_acc.tile([VW, CHUNK], f32, tag="acc")

                    def emit_st(j):
                        kt = k_sb[:, j * 128 : (j + 1) * 128]
                        squad = ps_s.tile([128, 2 * CHUNK], f32, tag="squad")
                        nc.tensor.matmul(
                            squad[:, 0:CHUNK], kt, q_sb[:, sl0], start=True, stop=True
                        )
                        nc.tensor.matmul(
                            squad[:, CHUNK:], kt, q_sb[:, sl1], start=True, stop=True
                        )
                        return squad

                    def emit_exp_act(squad, et_t, jj):
                        nc.scalar.activation(
                            out=et_t[:, jj, :], in_=squad[:], func=Exp,
                            scale=SCALE, bias=mshift[:],
                        )

                    def emit_exp_dve(squad, et_t, jj):
                        nc.vector.tensor_scalar(
                            out=et_t[:, jj, :].bitcast(u8), in0=squad[:],
                            scalar1=SCH_A, scalar2=SCH_B,
                            op0=mybir.AluOpType.mult, op1=mybir.AluOpType.add,
                        )

                    def emit_ev(jp, et_t):
                        # emission order is 1,2,...,15,0: pair 0 is deferred to
                        # the end of the cp so the first EV (start=True, on
                        # pair 1) no longer waits on the previous cp's acc
                        # evacuation at the head of the in-order PE queue
                        vpair = v_sb[:, jp * 2 * VW : (jp + 1) * 2 * VW].rearrange(
                            "p (two f) -> p two f", two=2
                        )
                        nc.tensor.matmul(
                            acc0[:], vpair, et_t[:, :, 0:CHUNK],
                            start=(jp == 0), stop=(jp == NKT // 2 - 1),
                            skip_group_check=True, perf_mode=DR,
                        )
                        nc.tensor.matmul(
                            acc1[:], vpair, et_t[:, :, CHUNK:],
                            start=(jp == 0), stop=(jp == NKT // 2 - 1),
                            skip_group_check=True, perf_mode=DR,
                        )


                    # software-pipelined emission: EV(p) is deferred until
                    # after S(2p+3), so by the time the (in-order) PE queue
                    # reaches it, the pair's exps have had ~2 S-tiles of
                    # engine time to finish -- PE never head-of-line blocks
                    # on the slower ACT/DVE exp stage.
                    ets = {}
                    epi = {}
                    for j in range(NKT):
                        p, jj = j // 2, j % 2
                        if cp < NCHUNK // 2 - 1 and j == 0:
                            emit_q(2 * cp + 2, pair=True)
                        if pending is not None:
                            if j == EPI_J[0]:
                                epi["tbank"] = epi_transposes(pending[1])
                            elif j == EPI_J[1]:
                                epi["onorm"] = epi_norm(epi.pop("tbank"))
                            elif j == EPI_J[2]:
                                epi_proj(pending[0], 0, epi["onorm"], act_assist=True)
                            elif j == EPI_J[3]:
                                epi_proj(pending[0], 1, epi.pop("onorm"),
                                         act_assist=OSB1_ACT)
                        sq = emit_st(j)
                        if jj == 0:
                            pool_e = e0pool if p == 0 else epool
                            et_t = pool_e.tile([128, 2, 2 * CHUNK], f8, tag="et")
                            ets[p] = et_t
                        if (j * DVE_N) % NKT < DVE_N and j % NKT not in EXTRA_ACT:
                            emit_exp_dve(sq, ets[p], jj)
                        else:
                            emit_exp_act(sq, ets[p], jj)
                        if j >= 3 and jj == 1:
                            pd = (j - 3) // 2
                            emit_ev(pd, ets.pop(pd))
                    emit_ev(NKT // 2 - 1, ets.pop(NKT // 2 - 1))
                    # evacuate acc promptly (frees the single acc psum slot)
                    acc_sb = episb.tile([D1, 2 * CHUNK], f32r, tag="accsb")
                    if ACC0_ACT:
                        nc.scalar.copy(acc_sb[:, 0:CHUNK], acc0[0:D1, :])
                    else:
                        nc.vector.tensor_copy(acc_sb[:, 0:CHUNK], acc0[0:D1, :])
                    nc.scalar.copy(acc_sb[:, CHUNK:], acc1[0:D1, :])
                    pending = (cp, acc_sb)
                tbank = epi_transposes(pending[1])
                onorm = epi_norm(tbank)
                epi_proj(pending[0], 0, onorm, act_assist=True)
                epi_proj(pending[0], 1, onorm)

    nc.compile()
    return nc


def _get_program():
    if "nc" not in _CACHE:
        _CACHE["nc"] = _build()
    return _CACHE["nc"]


def _make_in_maps(x, w_qkv, b_qkv, w_proj, b_proj):
    import ml_dtypes

    x_cl = np.ascontiguousarray(
        np.asarray(x, dtype=np.float32).reshape(C, L).astype(ml_dtypes.bfloat16)
    )
    w_qkv = np.asarray(w_qkv, dtype=np.float32)
    b_qkv = np.asarray(b_qkv, dtype=np.float32)
    w_proj = np.asarray(w_proj, dtype=np.float32)
    b_proj = np.asarray(b_proj, dtype=np.float32)

    wpT = np.ascontiguousarray(w_proj.T)  # (w, w_new)

    in_maps = []
    for i in range(N_CORES):
        rows_q = np.arange(D) * 24 + i * 3 + 0  # d-major split of the 3C axis
        rows_k = rows_q + 1
        rows_v = rows_q + 2
        cb = np.zeros((C, CBLOB_W), dtype=np.float32)
        cb[:, 50:178] = wpT
        cb[0:D, 178] = b_qkv[rows_q]  # bq
        cb[0:D, 179] = b_qkv[rows_k]  # bk
        cb[:, 180] = -SHIFT  # ACT exp bias column
        cb[0, 198:326] = b_proj
        cb[0:D1, 326:344] = np.eye(D1, dtype=np.float32)
        cb[0, 344:472] = 1.0  # ones128
        cb[0, 472:488] = 1.0  # ones16
        cb[0, 488:616] = b_proj
        cb[0, 616:744] = b_proj
        wkvb = np.zeros((W, W + 2 * D + VW + 204), dtype=ml_dtypes.bfloat16)
        wkvb[0, W + 2 * VW + 64 : W + 2 * VW + 192] = 1.0  # bf16 ones row
        # f32 bias values byte-packed into pairs of bf16 columns
        wkvb[0:D, 384:386] = b_qkv[rows_q].astype(np.float32).reshape(D, 1).view(ml_dtypes.bfloat16)
        wkvb[0:D, 386:388] = b_qkv[rows_k].astype(np.float32).reshape(D, 1).view(ml_dtypes.bfloat16)
        wkvb[:, 388:390] = np.full((W, 1), -SHIFT, np.float32).view(ml_dtypes.bfloat16)
        wkvb[:, 0:W] = wpT.astype(ml_dtypes.bfloat16)
        # bv row: [1 | bv | 0]
        wkvb[0, W] = 1.0
        wkvb[0, W + 1 : W + 1 + D] = b_qkv[rows_v].astype(ml_dtypes.bfloat16)
        # wv block: [0 | wv | 0] (col 0 zero so the ones col stays exact)
        wkvb[:, W + VW + 1 : W + VW + 1 + D] = w_qkv[rows_v].T.astype(
            ml_dtypes.bfloat16
        )
        base = W + 2 * VW
        wkvb[:, base : base + D] = w_qkv[rows_q].T.astype(ml_dtypes.bfloat16)
        wkvb[:, base + 32 : base + 32 + D] = w_qkv[rows_k].T.astype(
            ml_dtypes.bfloat16
        )
        in_maps.append({"x_cl": x_cl, "cblob": cb, "wpbf": wkvb})
    return in_maps


def _run(in_maps, trace=False):
    from concourse.bass_utils import run_bass_kernel_spmd

    nc = _get_program()
    return run_bass_kernel_spmd(nc, in_maps, list(range(N_CORES)), trace=trace)


def _assemble(results):
    out = np.empty((1, C, H, W), dtype=np.float32)
    for i in range(N_CORES):
        out[0, i * D : (i + 1) * D] = results[i]["out"].reshape(D, H, W)
    return out


def kernel(x, w_qkv, b_qkv, w_proj, b_proj):
    in_maps = _make_in_maps(x, w_qkv, b_qkv, w_proj, b_proj)
    r = _run(in_maps, trace=False)
    return _assemble(r.results)


def kernel_with_timing(x, w_qkv, b_qkv, w_proj, b_proj):
    """Like kernel() but also returns an HW execution time estimate in ns.

    The axon client in this container has no NTFF profiling hook, so when
    hardware profiling is unavailable we fall back to the concourse
    cost-model timeline simulator (single core; cores are identical/independent).
    """
    in_maps = _make_in_maps(x, w_qkv, b_qkv, w_proj, b_proj)
    try:
        r = _run(in_maps, trace=True)
        exec_ns = r.exec_time_ns
    except ModuleNotFoundError:
        r = _run(in_maps, trace=False)
        exec_ns = None
    if exec_ns is None:
        exec_ns = _CACHE.get("tlsim_ns")
        if exec_ns is None:
            from concourse.timeline_sim import TimelineSim

            exec_ns = int(TimelineSim(_get_program()).simulate())
            _CACHE["tlsim_ns"] = exec_ns
    return _assemble(r.results), exec_ns
